# revision 6
# baseline (speedup 1.0000x reference)
"""Trainium2 Bass kernel for nn_CovBlock (B=4, N=8192, D=2048, H=512, F=64).

Data-parallel over 8 NeuronCores: x sharded along N (1024 rows/batch/core).
Main loop streams x (sync HWDGE ring, 2MB chunks), per 128-row tile:
DVE row-sum -> ACT scale + Square(bias=-mu) -> bf16 -> TensorE ones-column
matmul accumulating the per-batch column sum-of-squares in PSUM [1, D].

Tail is a per-batch staged pipeline so the collectives for batches 0..2
overlap the main loop (stage A: evac+bounce+AllGather right after a batch's
last tile; stage B: DMA-transpose + cov + L1 + L2-partial + AllGather2 one
batch later; stage C: DMA-transpose + reduce + bias + leaky + L3 two
batches later).  Collectives carry bf16; HWDGE DMA-transpose replaces PE
transposes and the gather+reduce hop.  Stage DMAs ride the scalar (ACT)
HWDGE ring so they never block the x stream on the sync ring.
"""

import sys

sys.path.insert(0, "/opt/trn_rl_repo")

import numpy as np

B, N, D, H, F = 4, 8192, 2048, 512, 64
NCORES = 8
P = 128
EPS = 1e-6
SLOPE = 0.01

_CACHE = {}


def _build(nsh, debug=False, chunk_tiles=2, xbufs=5, sqbufs=5):
    import concourse.bacc as bacc
    import concourse.mybir as mybir
    from concourse import tile

    dt = mybir.dt.float32
    bt = mybir.dt.bfloat16
    AF = mybir.ActivationFunctionType
    ROWS = B * nsh
    NT = ROWS // P            # total 128-row tiles per core
    TPB = nsh // P            # tiles per batch (8)
    KC = D // P               # 16 k-chunks of 128
    JSL = D // NCORES         # 256: L1 output column slice per core
    J2C = JSL // P            # 2:  L1-slice k-chunks for L2
    HC = H // P               # 4:  H chunks of 128
    CT = min(chunk_tiles, NT)
    NCH = NT // CT            # chunks total
    CPB = TPB // CT           # chunks per batch
    assert NT % CT == 0 and nsh % P == 0 and TPB % CT == 0

    nc = bacc.Bacc("TRN2", target_bir_lowering=False, debug=False,
                   num_devices=NCORES)

    x = nc.dram_tensor("x", [ROWS, D], dt, kind="ExternalInput")
    w1t = nc.dram_tensor("w1t", [P, KC, JSL], bt, kind="ExternalInput")
    w2t = nc.dram_tensor("w2t", [P, J2C, H], bt, kind="ExternalInput")
    w3t = nc.dram_tensor("w3t", [P, HC, F], bt, kind="ExternalInput")
    b1r = nc.dram_tensor("b1r", [1, JSL], bt, kind="ExternalInput")
    b2tin = nc.dram_tensor("b2tin", [P, HC], dt, kind="ExternalInput")
    b3r = nc.dram_tensor("b3r", [1, F], bt, kind="ExternalInput")
    out = nc.dram_tensor("out", [B, F], dt, kind="ExternalOutput")
    dbg = {}
    if debug:
        dbg["dbg_ssum"] = nc.dram_tensor("dbg_ssum", [P, KC * B], dt,
                                         kind="ExternalOutput")
        dbg["dbg_cov"] = nc.dram_tensor("dbg_cov", [P, KC * B], dt,
                                        kind="ExternalOutput")

    groups = [list(range(NCORES))]

    with tile.TileContext(nc) as tc:
        with (
            tc.tile_pool(name="xp", bufs=xbufs) as xp,
            tc.tile_pool(name="sq", bufs=sqbufs) as sq,
            tc.tile_pool(name="sm", bufs=12) as sm,
            tc.tile_pool(name="wp", bufs=1) as wp,
            tc.tile_pool(name="tl", bufs=1) as tl,
            tc.tile_pool(name="gp", bufs=2) as gp,
            tc.tile_pool(name="pp", bufs=1, space="PSUM") as pp,
            tc.tile_pool(name="dr", bufs=1, space="DRAM") as dr,
        ):
            # ---- first x chunk DMA before anything else ----
            xch = [None] * NCH
            xch[0] = xp.tile([P, CT, D], dt, name="xch")
            nc.sync.dma_start(
                xch[0][:],
                x.ap()[0:CT * P, :].rearrange("(t p) d -> p t d", p=P))

            # constants
            ones128 = wp.tile([P, 1], bt)
            nc.any.memset(ones128[:], 1.0)
            ones11 = wp.tile([1, 1], bt)
            nc.any.memset(ones11[:], 1.0)
            ident1f = wp.tile([1, 1], dt)
            nc.any.memset(ident1f[:], 1.0)

            # weight/bias prefetch on the SWDGE (gpsimd) ring
            w1sb = wp.tile([P, KC, JSL], bt)
            w2sb = wp.tile([P, J2C, H], bt)
            w3sb = wp.tile([P, HC, F], bt)
            b1row = wp.tile([1, JSL], bt)
            b2T = wp.tile([P, HC], dt)
            b3row = wp.tile([1, F], bt)
            nc.gpsimd.dma_start(w1sb[:], w1t.ap()[:, :, :])
            nc.gpsimd.dma_start(w2sb[:], w2t.ap()[:, :, :])
            nc.gpsimd.dma_start(w3sb[:], w3t.ap()[:, :, :])
            nc.gpsimd.dma_start(b1row[:], b1r.ap()[:, :])
            nc.gpsimd.dma_start(b2T[:], b2tin.ap()[:, :])
            nc.gpsimd.dma_start(b3row[:], b3r.ap()[:, :])

            # per-batch DRAM tiles for the two collectives
            ss_in = [dr.tile([1, D], bt, name=f"ss_in{b}") for b in range(B)]
            ss_g = [dr.tile([NCORES, D], bt, name=f"ss_g{b}", addr_space="Shared") for b in range(B)]
            h2_in = [dr.tile([1, H], bt, name=f"h2_in{b}") for b in range(B)]
            h2_g = [dr.tile([NCORES, H], bt, name=f"h2_g{b}", addr_space="Shared") for b in range(B)]

            state = {}

            def stage_a(b):
                # evacuate batch-b ss row (bf16) -> DRAM -> AllGather
                ssrow = sm.tile([1, D], bt, tag="ssrow", bufs=2)
                nc.scalar.copy(ssrow[:], state[("ss", b)][:])
                nc.scalar.dma_start(ss_in[b][:], ssrow[:])
                nc.gpsimd.collective_compute(
                    "AllGather", mybir.AluOpType.bypass,
                    replica_groups=groups,
                    ins=[ss_in[b].opt()], outs=[ss_g[b].opt()])

            def stage_b(b):
                # gathered [8, D] -> (transpose DMA) [P, 8*KC] -> cov -> L1
                # -> leaky -> h1T -> L2 partial -> AllGather2
                gT = gp.tile([P, NCORES * KC], bt, tag="gT")
                nc.scalar.dma_start_transpose(
                    gT[:],
                    ss_g[b].opt().rearrange("i (c p) -> (i c) p", p=P))
                ssum = sm.tile([P, KC], dt, tag="ssum", bufs=2)
                nc.vector.reduce_sum(
                    ssum[:], gT[:].rearrange("p (i c) -> p c i", i=NCORES),
                    axis=mybir.AxisListType.X)
                t1 = sm.tile([P, KC], dt, tag="t1", bufs=2)
                nc.vector.tensor_scalar_add(t1[:], ssum[:], EPS)
                t2 = sm.tile([P, KC], dt, tag="t2", bufs=2)
                nc.vector.reciprocal(t2[:], t1[:])
                cov = sm.tile([P, KC], bt, tag="cov", bufs=2)
                nc.vector.tensor_mul(cov[:], ssum[:], t2[:])
                if debug:
                    nc.vector.tensor_copy(
                        state.setdefault(
                            ("dbg_ssum_sb",),
                            tl.tile([P, KC * B], dt, name="dbg_ssum_sb"))[:, b * KC:(b + 1) * KC],
                        ssum[:])
                    nc.vector.tensor_copy(
                        state.setdefault(
                            ("dbg_cov_sb",),
                            tl.tile([P, KC * B], dt, name="dbg_cov_sb"))[:, b * KC:(b + 1) * KC],
                        cov[:])

                h1p = pp.tile([1, JSL], dt, tag="tps", bufs=2)
                for c in range(KC):
                    nc.tensor.matmul(h1p[:], lhsT=cov[:, c:c + 1],
                                     rhs=w1sb[:, c, :],
                                     start=(c == 0), stop=False)
                nc.tensor.matmul(h1p[:], lhsT=ones11[:], rhs=b1row[:],
                                 start=False, stop=True)
                h1a = sm.tile([1, JSL], dt, tag="h1a", bufs=2)
                nc.vector.tensor_scalar_mul(h1a[:], h1p[:], SLOPE)
                h1s = sm.tile([1, JSL], dt, tag="h1s", bufs=2)
                nc.vector.tensor_max(h1s[:], h1p[:], h1a[:])

                h1T_psum = pp.tile([P, J2C], dt, tag="tps", bufs=2)
                for cc in range(J2C):
                    nc.tensor.transpose(h1T_psum[:, cc:cc + 1],
                                        h1s[0:1, cc * P:(cc + 1) * P],
                                        ident1f[:])
                h1T = sm.tile([P, J2C], bt, tag="h1T", bufs=2)
                nc.vector.tensor_copy(h1T[:], h1T_psum[:])

                h2p = pp.tile([1, H], dt, tag="tps", bufs=2)
                for cc in range(J2C):
                    nc.tensor.matmul(h2p[:], lhsT=h1T[:, cc:cc + 1],
                                     rhs=w2sb[:, cc, :],
                                     start=(cc == 0), stop=(cc == J2C - 1))
                h2row = sm.tile([1, H], bt, tag="h2row", bufs=2)
                nc.scalar.copy(h2row[:], h2p[:])
                nc.scalar.dma_start(h2_in[b][:], h2row[:])
                nc.gpsimd.collective_compute(
                    "AllGather", mybir.AluOpType.bypass,
                    replica_groups=groups,
                    ins=[h2_in[b].opt()], outs=[h2_g[b].opt()])

            def stage_c(b):
                # gathered [8, H] -> (transpose DMA) [P, 8*HC] -> reduce
                # -> +b2 -> leaky -> L3 -> out row
                g2T = gp.tile([P, NCORES * HC], bt, tag="g2T")
                nc.scalar.dma_start_transpose(
                    g2T[:],
                    h2_g[b].opt().rearrange("i (c p) -> (i c) p", p=P))
                h2pre = sm.tile([P, HC], dt, tag="h2pre", bufs=2)
                nc.vector.reduce_sum(
                    h2pre[:], g2T[:].rearrange("p (i c) -> p c i", i=NCORES),
                    axis=mybir.AxisListType.X)
                h2b = sm.tile([P, HC], dt, tag="h2b", bufs=2)
                nc.vector.tensor_add(h2b[:], h2pre[:], b2T[:])
                h2a = sm.tile([P, HC], dt, tag="h2a", bufs=2)
                nc.vector.tensor_scalar_mul(h2a[:], h2b[:], SLOPE)
                h2T = sm.tile([P, HC], bt, tag="h2T", bufs=2)
                nc.vector.tensor_max(h2T[:], h2b[:], h2a[:])

                outp = pp.tile([1, F], dt, tag="tps", bufs=2)
                for r in range(HC):
                    nc.tensor.matmul(outp[:], lhsT=h2T[:, r:r + 1],
                                     rhs=w3sb[:, r, :],
                                     start=(r == 0), stop=False)
                nc.tensor.matmul(outp[:], lhsT=ones11[:], rhs=b3row[:],
                                 start=False, stop=True)
                outrow = sm.tile([1, F], dt, tag="outrow", bufs=2)
                nc.vector.tensor_copy(outrow[:], outp[:])
                nc.scalar.dma_start(out.ap()[b:b + 1, :], outrow[:])

            # ---- main pass over x ----
            for k in range(NCH):
                if k > 0:
                    xch[k] = xp.tile([P, CT, D], dt, name="xch")
                    src = x.ap()[k * CT * P:(k + 1) * CT * P, :]
                    nc.sync.dma_start(
                        xch[k][:], src.rearrange("(t p) d -> p t d", p=P))
                for t in range(CT):
                    g = k * CT + t
                    b, tib = g // TPB, g % TPB
                    if tib == 0:
                        state[("ss", b)] = pp.tile([1, D], dt, tag="ss",
                                                   bufs=1, name="ssb")
                    xt = xch[k][:, t, :]
                    negsum = sm.tile([P, 1], dt, tag="negsum", bufs=6)
                    nc.vector.reduce_sum(negsum[:], xt,
                                         axis=mybir.AxisListType.X)
                    negmu = sm.tile([P, 1], dt, tag="negmu", bufs=6)
                    nc.scalar.mul(negmu[:], negsum[:], -1.0 / D)
                    xsq = sq.tile([P, D], bt)
                    nc.scalar.activation(xsq[:], xt, AF.Square,
                                         bias=negmu[:], scale=1.0)
                    for q in range(D // 512):
                        nc.tensor.matmul(
                            state[("ss", b)][:, q * 512:(q + 1) * 512],
                            lhsT=ones128[:],
                            rhs=xsq[:, q * 512:(q + 1) * 512],
                            start=(tib == 0), stop=(tib == TPB - 1))
                    if tib == TPB - 1:
                        stage_a(b)
                        if b >= 1:
                            stage_b(b - 1)
                        if b >= 2:
                            stage_c(b - 2)

            # ---- drain the pipeline ----
            stage_c(B - 2)
            stage_b(B - 1)
            stage_c(B - 1)
            if debug:
                nc.scalar.dma_start(dbg["dbg_ssum"].ap()[:, :],
                                    state[("dbg_ssum_sb",)][:])
                nc.scalar.dma_start(dbg["dbg_cov"].ap()[:, :],
                                    state[("dbg_cov_sb",)][:])

    nc.compile()
    return nc


def _get_nc(nsh=N // NCORES, debug=False):
    key = (nsh, debug)
    if key not in _CACHE:
        _CACHE[key] = _build(nsh, debug=debug)
    return _CACHE[key]


def _bf(a):
    import ml_dtypes
    return np.ascontiguousarray(a).astype(ml_dtypes.bfloat16)


def make_in_maps(x, W1, b1, W2, b2, W3, b3, nsh=N // NCORES):
    JSL = D // NCORES
    KC, J2C, HC = D // P, JSL // P, H // P
    x = np.asarray(x, dtype=np.float32)
    W1 = np.asarray(W1, dtype=np.float32)
    b1 = np.asarray(b1, dtype=np.float32)
    W2 = np.asarray(W2, dtype=np.float32)
    b2 = np.asarray(b2, dtype=np.float32)
    W3 = np.asarray(W3, dtype=np.float32)
    b3 = np.asarray(b3, dtype=np.float32)
    w3t = _bf(W3.reshape(HC, P, F).transpose(1, 0, 2))
    b2t = np.ascontiguousarray(b2.reshape(HC, P).T)
    in_maps = []
    for i in range(NCORES):
        xs = np.ascontiguousarray(
            x[:, i * nsh:(i + 1) * nsh, :]).reshape(B * nsh, D)
        w1s = W1[:, i * JSL:(i + 1) * JSL]
        w2s = W2[i * JSL:(i + 1) * JSL, :]
        in_maps.append({
            "x": xs,
            "w1t": _bf(w1s.reshape(KC, P, JSL).transpose(1, 0, 2)),
            "w2t": _bf(w2s.reshape(J2C, P, H).transpose(1, 0, 2)),
            "w3t": w3t,
            "b1r": _bf(b1[i * JSL:(i + 1) * JSL]).reshape(1, JSL),
            "b2tin": b2t, "b3r": _bf(b3).reshape(1, F),
        })
    return in_maps


def run(x, W1, b1, W2, b2, W3, b3, nsh=N // NCORES, debug=False, trace=False):
    from concourse.bass_utils import run_bass_kernel_spmd
    nc = _get_nc(nsh, debug)
    in_maps = make_in_maps(x, W1, b1, W2, b2, W3, b3, nsh=nsh)
    res = run_bass_kernel_spmd(nc, in_maps, list(range(NCORES)), trace=trace)
    return res


def kernel(x, W1, b1, W2, b2, W3, b3):
    res = run(x, W1, b1, W2, b2, W3, b3)
    return np.asarray(res.results[0]["out"], dtype=np.float32)
